# revision 50
# baseline (speedup 1.0000x reference)
"""Trainium2 Bass kernel for per-attribute MLP decoder (nn_AttrDecoder).

Computes, for each attribute a (A=312 independent blocks):
    h = relu(x[:, a*64:(a+1)*64] @ W1[a] + b1[a])      # [B, 128]
    o[:, a] = sigmoid(h @ W2[a] + b2[a])               # [B, 1]

Design notes (~235us vs the 292us v1 baseline; measured on HW):
  - The binding constraint is the PSUM->SBUF relu pass over h (40.9M
    elems/core): only ACT and DVE can read PSUM, at 1 elem/cycle/lane
    for fp32 sources, so that stage floors at ~200us+overheads across
    both engines.  Everything else is arranged to stay under it.
  - v1 burned ~86us of ACT on sigmoids over [128,1024] tiles with only
    4 live partitions (one per PE column-group).  Here o is packed
    densely: MM2 for attr a uses an M=32 one-hot stationary (W2[a] in
    column (a%128)//4, zeros elsewhere) accumulated onto a shared PSUM
    bank pair at tile_position (0, 32*(a%4)), so 128 attrs land on 128
    distinct partitions (start=True only opens each chain: it clears
    has_written for the written region only).  Sigmoid+b2 runs once per
    128-attr group (3 ops, ~3us) with b2 as a per-partition bias; the
    host undoes the partition permutation during the gather.
  - PSUM budget (8 banks) is the whole game: dense-o pins 2 banks for
    its accumulator, leaving 6 for h.  With 2-bank h tiles (FD=1024
    relu) the 3-deep pool created a latency cycle -- the next pair's
    MM1 waits on the CURRENT pair's ~1.1us relu to free a slot --
    pacing everything at ~1.7us/pair.  Single-bank h tiles (one per
    (attr, bt), 6-deep pool, one FD=512 relu per engine per attr,
    separate SBUF output tiles) cost ~7% extra relu overhead but let
    both halves run concurrently and free banks in ~0.7us, so the
    pipeline is relu-THROUGHPUT-paced (~1.45us/pair).
  - Deep downstream pools matter: relu outputs rotate through 40 SBUF
    tiles so their anti-deps (on MM2 reads ~2 quads back) never bind.
  - x streams in staged blocks (256KB first so MM1 starts ~9us in, then
    3MB = 24KB contiguous per partition) at near line rate; weights ride
    the gpsimd SWDGE queue in priority-ordered chunks.
  - relu halves split ACT:DVE = 321:303 to balance effective per-op
    rates (687ns vs 742ns at FD=512).
"""

import numpy as np
import ml_dtypes

import concourse.bass as bass
import concourse.tile as tile
from concourse import mybir
from concourse import bass_utils

A = 312
LAT = 64
HID = 128
B = 8192
NCORES = 8
BS = B // NCORES          # 1024 batch rows per core
NPAIR = A // 2            # 156
NQUAD = A // 4            # 78
BT = 512                  # batch tile (one PSUM bank of fp32)
NBT = BS // BT            # 2
XBLK = 12                 # pairs per x DMA (3 MB, 24 KB/partition)
NGRP = (A + 127) // 128   # 3 output groups (128 attrs dense per group)
N_ACT = 167               # relu ops assigned to ScalarE (rest on DVE)

_cached = {}


def _legalize_waits(nc, max_waits=1):
    """Walrus in this toolchain encodes at most one sync-wait per instruction.
    Hoist extra waits onto standalone EventSemaphore instructions placed just
    before the owner on the same engine queue (queue order preserves the
    happens-before)."""
    nsplit = 0
    for bb in nc.m.functions[0].blocks:
        new_insts = []
        changed = False
        for inst in bb.instructions:
            si = getattr(inst, "sync_info", None)
            if si is not None and len(si.on_wait) > max_waits:
                waits = list(si.on_wait)
                for k, w in enumerate(waits[:-max_waits]):
                    es = mybir.InstEventSemaphore(name=f"{inst.name}-hw{k}")
                    es.engine = inst.engine
                    es.opcode = "EventSemaphore"
                    es.sync_info = mybir.SyncInfo(on_wait=[w], on_update=[])
                    new_insts.append(es)
                    nsplit += 1
                inst.sync_info = mybir.SyncInfo(
                    on_wait=waits[-max_waits:], on_update=list(si.on_update))
                changed = True
            new_insts.append(inst)
        if changed:
            bb.instructions = new_insts
    return nsplit


def _build_nc():
    nc = bass.Bass("TRN2", target_bir_lowering=False, debug=False,
                   num_devices=NCORES)
    # xs[r, q, :] = x^T[q*128 + r, :] so each pair q is one partition-dim
    # slice and per-partition reads are contiguous across consecutive pairs.
    xs = nc.dram_tensor("xs", [128, NPAIR, BS], mybir.dt.bfloat16,
                        kind="ExternalInput").ap()
    w1 = nc.dram_tensor("w1", [128, NPAIR, 128], mybir.dt.bfloat16,
                        kind="ExternalInput").ap()
    # One-hot padded W2: w2oh[:, a, i] = W2[a] if i == (a%128)//4 else 0.
    w2 = nc.dram_tensor("w2", [HID, A, 32], mybir.dt.bfloat16,
                        kind="ExternalInput").ap()
    b1 = nc.dram_tensor("b1", [HID, A], mybir.dt.float32,
                        kind="ExternalInput").ap()
    # b2 permuted to the dense-o partition order: b2g[p, g] = b2[g*128+m]
    # with p = 32*(m%4) + m//4.
    b2 = nc.dram_tensor("b2", [128, NGRP], mybir.dt.float32,
                        kind="ExternalInput").ap()
    ot = nc.dram_tensor("ot", [NGRP * 128, BS], mybir.dt.float32,
                        kind="ExternalOutput").ap()

    with tile.TileContext(nc, trace_sim=False) as tc:
        _body(tc, xs, w1, w2, b1, b2, ot)
    _legalize_waits(nc)
    return nc


def _body(tc, xs, w1, w2, b1, b2, ot):
    nc = tc.nc
    from contextlib import ExitStack
    with ExitStack() as ctx:
        singles = ctx.enter_context(tc.tile_pool(name="singles", bufs=1))
        xpool = ctx.enter_context(tc.tile_pool(name="x", bufs=4))
        hsb = ctx.enter_context(tc.tile_pool(name="hsb", bufs=40))
        osb = ctx.enter_context(tc.tile_pool(name="osb", bufs=2))
        hps = ctx.enter_context(
            tc.tile_pool(name="hps", bufs=6, space=bass.MemorySpace.PSUM))
        ops = ctx.enter_context(
            tc.tile_pool(name="ops", bufs=1, space=bass.MemorySpace.PSUM))

        b1_sb = singles.tile([HID, A], mybir.dt.float32)
        w2_sb = singles.tile([HID, A, 32], mybir.dt.bfloat16)
        b2_sb = singles.tile([128, NGRP], mybir.dt.float32)
        w1_sb = singles.tile([128, NPAIR, 128], mybir.dt.bfloat16)

        # Resident weights: ship what the first pairs need on the fast sync
        # HWDGE queue, bulk follows on the gpsimd SWDGE queue in growing
        # chunks (issue overhead ~1us per SWDGE dma).
        nc.sync.dma_start(w1_sb[:, 0:8, :], w1[:, 0:8, :])
        nc.gpsimd.dma_start(b1_sb[:, 0:64], b1[:, 0:64])
        nc.gpsimd.dma_start(w1_sb[:, 8:20, :], w1[:, 8:20, :])
        nc.gpsimd.dma_start(w2_sb[:, 0:32, :], w2[:, 0:32, :])
        nc.gpsimd.dma_start(b2_sb[:], b2[:])
        nc.gpsimd.dma_start(b1_sb[:, 64:A], b1[:, 64:A])
        nc.gpsimd.dma_start(w2_sb[:, 32:128, :], w2[:, 32:128, :])
        for c, ce in [(20, 36), (36, 60), (60, 90), (90, 124), (124, 156)]:
            nc.gpsimd.dma_start(w1_sb[:, c:ce, :], w1[:, c:ce, :])
        nc.gpsimd.dma_start(w2_sb[:, 128:A, :], w2[:, 128:A, :])

        # Single-bank h tiles (one per (attr, bt), 6-deep pool): each FD=512
        # relu half frees its bank in ~0.7us and the two halves of an attr
        # run concurrently on ACT and DVE (different PSUM banks, separate
        # SBUF tiles).  Costs ~7% relu overhead vs one 1024-el op but
        # removes the latency coupling that paced 2-bank-tile variants at
        # ~1.7us/pair.  4 relu ops/pair (2 per engine) keeps enough
        # independent work queued per engine to absorb PE jitter (a
        # bt-major restructure with 1 op/engine/pass measured WORSE).
        NHALF = 2 * A
        N_ACT_HALF = 321
        act_half = [((k + 1) * N_ACT_HALF) // NHALF > (k * N_ACT_HALF) // NHALF
                    for k in range(NHALF)]

        o_ps = [None]       # current group's dense o accumulator

        def mm2_half(k, bt, quad):
            """One bt-half of quad k = attrs 4k..4k+3 -> col-groups 0..3
            (concurrent streams), accumulating into o_ps at chain position
            i=k%32.  Emitted one half per pair so the PE queue sees a
            smooth 4-matmul burst per pair instead of 8 every other pair."""
            i = k % 32
            stop = (i == 31) or (k == NQUAD - 1)
            if i == 0 and bt == 0:
                o_ps[0] = ops.tile([128, NBT, BT], mybir.dt.float32,
                                   name="o_dense")
            for t, (a, h_bts) in enumerate(quad):
                # start=True clears has_written for the written region
                # only (verified on HW: bank-wide-clear would break the
                # other col-groups' chains), so each chain opens with it.
                nc.tensor.matmul(
                    o_ps[0][32 * t:32 * t + 32, bt, :],
                    w2_sb[:, a, :],
                    h_bts[bt],
                    start=(i == 0), stop=stop,
                    tile_position=(0, 32 * t),
                )

        def sigmoid_store(g):
            """Dense sigmoid + store for 128-attr group g."""
            o_out = osb.tile([128, NBT, BT], mybir.dt.float32, name="osb")
            nc.scalar.activation(
                out=o_out[:], in_=o_ps[0][:],
                func=mybir.ActivationFunctionType.Sigmoid,
                bias=b2_sb[:, g:g + 1], scale=1.0)
            nc.sync.dma_start(
                out=ot[g * 128:(g + 1) * 128, :].rearrange(
                    "p (n b) -> p n b", n=NBT),
                in_=o_out[:])

        # x block sizes: small leading blocks so MM1 starts ~12us earlier
        # (a 3MB first DMA would stall the whole pipeline on its tail)
        xblks = [1, 2, 3, 5, 7] + [10] * 13 + [8]
        assert sum(xblks) == NPAIR
        xstart = {}
        s = 0
        for bi, sz in enumerate(xblks):
            xstart[s] = (bi, sz)
            s += sz

        x_tile = [None]
        x_off = [0]
        pend = []           # (a, [h_bt0, h_bt1]) relu'd attrs not yet MM2'd
        sig_g = None        # group whose sigmoid/store is deferred
        mm2_state = [0, 0]  # (next quad, next bt half)
        skip_mm2 = [0]      # emission slots to skip (group-boundary slack)
        behind = [0]        # skipped slots to catch up later

        def emit_mm2():
            k, bt = mm2_state
            mm2_half(k, bt, pend[:4])
            if bt == 0:
                mm2_state[1] = 1
                return False
            mm2_state[0], mm2_state[1] = k + 1, 0
            return True     # quad complete -> caller pops pend

        for p in range(NPAIR):
            if p in xstart:
                bi, sz = xstart[p]
                x_tile[0] = xpool.tile([128, sz, BS],
                                       mybir.dt.bfloat16, name="xt")
                x_off[0] = p
                # ramp blocks alternate between the two HWDGE queues so
                # their issue doesn't FIFO-serialize behind the previous
                # transfer (measured ~5us MM1 stall at t~28us otherwise);
                # steady-state blocks stay off the busy ACT queue
                q = nc.scalar if (bi < 6 and bi % 2 == 1) else nc.sync
                q.dma_start(
                    out=x_tile[0][:],
                    in_=xs[:, p:p + sz, :])
            off = p - x_off[0]
            # deferred sigmoid + the oldest quad's next MM2 half-burst go
            # ahead of this pair's MM1s on the PE queue.  At a group
            # boundary, skip one emission slot so the sigmoid (whose read
            # the next group's first MM2 WARs on, o pool being 1-deep) has
            # a full pair of slack before the PE FIFO needs those banks --
            # otherwise the blocked MM2 head-blocks every MM1 behind it
            # and the relu engines bubble ~6us; catch the slot up later.
            if sig_g is not None and len(pend) >= 8:
                sigmoid_store(sig_g)
                sig_g = None
                skip_mm2[0] = 1
            if skip_mm2[0]:
                skip_mm2[0] = 0
                behind[0] += 1
            else:
                n_emit = 2 if behind[0] > 0 else 1
                if n_emit == 2:
                    behind[0] -= 1
                for _ in range(n_emit):
                    if len(pend) < 8:
                        break
                    k = mm2_state[0]
                    if emit_mm2():
                        if (k % 32 == 31) or k == NQUAD - 1:
                            sig_g = k // 32
                        pend = pend[4:]
            # one single-bank PSUM tile per (attr, bt); bt-outer so each
            # bt's j0/j1 streams pair on disjoint PE row groups
            h_ps = [[hps.tile([128, BT], mybir.dt.float32, name="hp")
                     for _ in range(NBT)] for _ in range(2)]
            for bt in range(NBT):
                for j in range(2):
                    nc.tensor.matmul(
                        h_ps[j][bt][:],
                        w1_sb[j * 64:(j + 1) * 64, p, :],
                        x_tile[0][j * 64:(j + 1) * 64, off,
                                  bass.ds(bt * BT, BT)],
                        start=True, stop=True,
                        tile_position=(j * 64, 0),
                    )
            # per-(attr, bt) relu halves, each its own SBUF tile, engine by
            # global balance; emitted in data-ready order
            h_out = [[None, None], [None, None]]
            for bt in range(NBT):
                for j in range(2):
                    a = 2 * p + j
                    h_sb = hsb.tile([HID, BT], mybir.dt.bfloat16, name="hh")
                    h_out[j][bt] = h_sb[:]
                    if act_half[2 * a + bt]:
                        nc.scalar.activation(
                            out=h_sb[:], in_=h_ps[j][bt][:],
                            func=mybir.ActivationFunctionType.Relu,
                            bias=b1_sb[:, a:a + 1], scale=1.0)
                    else:
                        nc.vector.tensor_scalar(
                            out=h_sb[:], in0=h_ps[j][bt][:],
                            scalar1=b1_sb[:, a:a + 1], scalar2=0.0,
                            op0=mybir.AluOpType.add,
                            op1=mybir.AluOpType.max)
            pend.append((2 * p, h_out[0]))
            pend.append((2 * p + 1, h_out[1]))
        while pend:
            if sig_g is not None:
                sigmoid_store(sig_g)
                sig_g = None
            k = mm2_state[0]
            if emit_mm2():
                if (k % 32 == 31) or k == NQUAD - 1:
                    sig_g = k // 32
                pend = pend[4:]
        sigmoid_store(sig_g)


def _install_ntff_hook():
    """Register the axon NTFF profile hook (normally provided by the agent
    image's antenv.axon_hooks). Needed only for trace=True runs."""
    import sys as _sys, types as _types, ctypes, contextlib

    if "antenv.axon_hooks" not in _sys.modules:
        mod = _types.ModuleType("antenv.axon_hooks")
        _h = [None]
        mod.set_axon_ntff_profile_hook = lambda h: _h.__setitem__(0, h)
        mod.get_axon_ntff_profile_hook = lambda: _h[0]
        _sys.modules["antenv.axon_hooks"] = mod
        try:
            import antenv
            antenv.axon_hooks = mod
        except ImportError:
            pass
    mod = _sys.modules["antenv.axon_hooks"]
    if mod.get_axon_ntff_profile_hook() is not None:
        return

    lib = ctypes.CDLL("/opt/axon/libaxon_pjrt.so")
    lib.axon_start_nrt_profile.argtypes = [
        ctypes.POINTER(ctypes.c_int64), ctypes.c_size_t]
    lib.axon_start_nrt_profile.restype = ctypes.c_int64
    lib.axon_stop_nrt_profile.argtypes = [ctypes.c_char_p]
    lib.axon_stop_nrt_profile.restype = ctypes.c_int64

    @contextlib.contextmanager
    def _hook(output_dir, device_ids):
        import jax
        jax.devices()
        if device_ids:
            ids = (ctypes.c_int64 * len(device_ids))(*device_ids)
            rc = lib.axon_start_nrt_profile(ids, len(device_ids))
        else:
            rc = lib.axon_start_nrt_profile(None, 0)
        if rc != 0:
            raise RuntimeError(f"axon_start_nrt_profile rc={rc}")
        try:
            yield
        finally:
            n = lib.axon_stop_nrt_profile(str(output_dir).encode())
            print(f"ntff profile: {n} file(s) -> {output_dir}")

    mod.set_axon_ntff_profile_hook(_hook)
    # artifact upload needs a bucket; stub it out for local profiling
    bass_utils.upload_artifacts = lambda tmpdir: f"local://{tmpdir}"


def kernel(x, W1, b1, W2, b2, trace=False):
    if "nc" not in _cached:
        _cached["nc"] = _build_nc()
    nc = _cached["nc"]
    if trace:
        try:
            _install_ntff_hook()
        except Exception as e:
            print("ntff hook install failed:", e)
            trace = False

    xt = np.ascontiguousarray(
        x.reshape(B, A * LAT).astype(ml_dtypes.bfloat16).T)     # [19968, 8192]
    w1h = np.ascontiguousarray(
        W1.reshape(NPAIR, 128, 128).transpose(1, 0, 2)).astype(
            ml_dtypes.bfloat16)                                  # [128,156,128]
    # one-hot padded W2: column (a%128)//4 of slot a holds W2[a]
    w2h = np.zeros((HID, A, 32), np.float32)
    cols = (np.arange(A) % 128) // 4
    w2h[:, np.arange(A), cols] = W2.reshape(A, HID).T
    w2h = w2h.astype(ml_dtypes.bfloat16)
    b1h = np.ascontiguousarray(b1.T).astype(np.float32)          # [128, 312]
    # b2 permuted to dense-o partition order
    b2h = np.zeros((128, NGRP), np.float32)
    for g in range(NGRP):
        m = np.arange(min(128, A - g * 128))
        b2h[32 * (m % 4) + m // 4, g] = b2.reshape(A)[g * 128 + m]

    in_maps = []
    for c in range(NCORES):
        xc = xt[:, c * BS:(c + 1) * BS]                          # [19968,1024]
        xsh = np.ascontiguousarray(
            xc.reshape(NPAIR, 128, BS).transpose(1, 0, 2))       # [128,156,1024]
        in_maps.append({
            "xs": xsh, "w1": w1h, "w2": w2h, "b1": b1h, "b2": b2h,
        })

    res = bass_utils.run_bass_kernel_spmd(
        nc, in_maps, core_ids=list(range(NCORES)), trace=trace)
    _cached["last_results"] = res

    # undo the dense-o partition permutation: attr a = g*128+m lives at
    # device row g*128 + 32*(m%4) + m//4
    aa = np.arange(A)
    g, m = aa // 128, aa % 128
    rows = g * 128 + 32 * (m % 4) + m // 4
    out = np.empty((B, A), np.float32)
    for c in range(NCORES):
        out[c * BS:(c + 1) * BS, :] = res.results[c]["ot"][rows, :].T
    return out
